# revision 18
# baseline (speedup 1.0000x reference)
"""AttentionLSTM Trainium2 kernel.

N=512, T=32, D=1024, H=1024. 8-way data parallel over batch (64 rows/core).

Per-core algorithm:
  Phase A: xW[(t,n), 4H] = x @ Wx + b  (big efficient matmul, staged to DRAM bf16)
  Loop t = 0..31:
    scores  : PE diag-trick  psum[n,(n',k)] = h @ Af_flat, mask+reduce -> [n,16]
    softmax : exp on ACT (scale 1/32), reshape via DRAM roundtrip to col layout,
              sums + reciprocal broadcast via tiny PE matmuls
    attn    : PE block-diagonal matmuls -> attnT [d, n] directly (transposed)
    v       : psum[n, j] accum of h@Wh + attn@Wattn (h/attn stationary), + xW add
    gates   : sigmoid/tanh on ACT from psum, c/h update on DVE
    hT      : PE transpose h -> stationary layout for next step
"""
import sys
import os

sys.path.insert(0, "/opt/trn_rl_repo")

import numpy as np
from ml_dtypes import bfloat16

N, T, D, H = 512, 32, 1024, 1024
NCORES = 8
NL = N // NCORES          # 64 rows per core
J = 4 * H                 # 4096
NDT = D // 128            # 8 contraction tiles
NG = NL // 8              # 8 attention groups of 8 samples
SCALE = 1.0 / (H ** 0.5)  # 1/32

_CACHE = {}


def _build():
    import concourse.bass as bass
    import concourse.mybir as mybir
    from concourse import tile

    f32 = mybir.dt.float32
    bf16 = mybir.dt.bfloat16
    AF = mybir.ActivationFunctionType
    AX = mybir.AxisListType
    OP = mybir.AluOpType

    nc = bass.Bass()

    # ---- external inputs (per-core) ----
    xT_in = nc.dram_tensor("xT", (NDT, 128, T * NL), bf16, kind="ExternalInput")
    wx_in = nc.dram_tensor("Wx", (NDT, 128, J), bf16, kind="ExternalInput")
    wh_in = nc.dram_tensor("Wh", (NDT, 128, J), bf16, kind="ExternalInput")
    wat_in = nc.dram_tensor("Wat", (NDT, 128, J), bf16, kind="ExternalInput")
    afsc_in = nc.dram_tensor("afsc", (NDT, 128, NL * 16), bf16, kind="ExternalInput")
    afbd_in = nc.dram_tensor("afbd", (NG * NDT, 128, 128), bf16, kind="ExternalInput")
    mdiag_in = nc.dram_tensor("mdiag", (NL, NL * 16), bf16, kind="ExternalInput")
    bdones_in = nc.dram_tensor("bdones", (128, 8), f32, kind="ExternalInput")
    bdonesT_in = nc.dram_tensor("bdonesT", (8, 128), f32, kind="ExternalInput")
    bdmask_in = nc.dram_tensor("bdmask", (128, NL), bf16, kind="ExternalInput")
    ident_in = nc.dram_tensor("ident", (NL, NL), f32, kind="ExternalInput")
    hT0_in = nc.dram_tensor("hT0", (128, NDT * NL), bf16, kind="ExternalInput")
    c0_in = nc.dram_tensor("c0", (NL, H), f32, kind="ExternalInput")
    bias_in = nc.dram_tensor("bias", (1, J), bf16, kind="ExternalInput")
    ones_in = nc.dram_tensor("onescol", (1, 128), bf16, kind="ExternalInput")

    hs_out = nc.dram_tensor("hs", (NL, T, H), f32, kind="ExternalOutput")

    with tile.TileContext(nc) as tc:
        with tc.tile_pool(name="dram", bufs=1, space="DRAM") as dpool:
            xw_dram = dpool.tile([T * NL, J], bf16)
            m_dram = dpool.tile([NL, 16], f32)

            # ---------------- Phase A: xW = x @ Wx + b ----------------
            with (
                tc.tile_pool(name="pa", bufs=1) as pa,
                tc.tile_pool(name="paps", bufs=4, space="PSUM") as paps,
                tc.tile_pool(name="pao", bufs=2) as pao,
            ):
                xTs = pa.tile([128, NDT * T * NL], bf16)     # 32KB/part
                wxs = pa.tile([128, NDT * J], bf16)          # 64KB/part
                bbs = pa.tile([1, J], bf16)
                ones = pa.tile([1, 128], bf16)
                for dt in range(NDT):
                    nc.sync.dma_start(xTs[:, dt * 2048:(dt + 1) * 2048], xT_in[dt])
                    nc.sync.dma_start(wxs[:, dt * J:(dt + 1) * J], wx_in[dt])
                nc.sync.dma_start(bbs[:], bias_in[:, :])
                nc.sync.dma_start(ones[:], ones_in[:, :])

                for m in range(T * NL // 128):   # 16 row-tiles of (t,n)
                    ob = pao.tile([128, J], bf16, tag="pao")
                    for jc in range(J // 512):   # 8
                        ps = paps.tile([128, 512], f32, tag="paps")
                        for dt in range(NDT):
                            nc.tensor.matmul(
                                ps[:],
                                xTs[:, dt * 2048 + m * 128: dt * 2048 + (m + 1) * 128],
                                wxs[:, dt * J + jc * 512: dt * J + (jc + 1) * 512],
                                start=(dt == 0), stop=False,
                            )
                        nc.tensor.matmul(
                            ps[:], ones[:], bbs[:, jc * 512:(jc + 1) * 512],
                            start=False, stop=True,
                        )
                        nc.scalar.copy(ob[:, jc * 512:(jc + 1) * 512], ps[:])
                    nc.gpsimd.dma_start(xw_dram[m * 128:(m + 1) * 128, :], ob[:])

            # ---------------- Main recurrent loop ----------------
            with (
                tc.tile_pool(name="wp", bufs=1) as wp,
                tc.tile_pool(name="st", bufs=1) as st,
                tc.tile_pool(name="hp", bufs=1) as hp,
                tc.tile_pool(name="cp", bufs=2) as cp,
                tc.tile_pool(name="tp", bufs=1) as tp,
                tc.tile_pool(name="xwp", bufs=2) as xwp,
                tc.tile_pool(name="vps", bufs=1, space="PSUM") as vpsp,
                tc.tile_pool(name="scps", bufs=1, space="PSUM") as scpsp,
                tc.tile_pool(name="atps", bufs=1, space="PSUM") as atpsp,
                tc.tile_pool(name="smps", bufs=1, space="PSUM") as smpsp,
            ):
                whs = wp.tile([128, NDT * J], bf16)      # 64KB/part
                wats = wp.tile([128, NDT * J], bf16)     # 64KB/part
                afsc = wp.tile([128, NDT * NL * 16], bf16)  # 16KB/part
                afbd = wp.tile([128, NG * NDT * 128], bf16)  # 16KB/part
                mdiag = wp.tile([NL, NL * 16], bf16)
                bdon = wp.tile([128, 8], f32)
                bdonT = wp.tile([8, 128], f32)
                bdm = wp.tile([128, NL], bf16)
                idnt = wp.tile([NL, NL], f32)

                for dt in range(NDT):
                    nc.sync.dma_start(whs[:, dt * J:(dt + 1) * J], wh_in[dt])
                    nc.sync.dma_start(wats[:, dt * J:(dt + 1) * J], wat_in[dt])
                    nc.sync.dma_start(afsc[:, dt * 1024:(dt + 1) * 1024], afsc_in[dt])
                for gd in range(NG * NDT):
                    nc.sync.dma_start(afbd[:, gd * 128:(gd + 1) * 128], afbd_in[gd])
                nc.sync.dma_start(mdiag[:], mdiag_in[:, :])
                nc.sync.dma_start(bdon[:], bdones_in[:, :])
                nc.sync.dma_start(bdonT[:], bdonesT_in[:, :])
                nc.sync.dma_start(bdm[:], bdmask_in[:, :])
                nc.sync.dma_start(idnt[:], ident_in[:, :])

                # initial state
                hT = st.tile([128, NDT * NL], bf16, tag="hT0")
                nc.sync.dma_start(hT[:], hT0_in[:, :])
                c_cur = cp.tile([NL, H], f32, tag="c2")
                nc.sync.dma_start(c_cur[:], c0_in[:, :])

                for t in range(T):
                    # ---- scores: psum[n, (n',k)] += hT.T @ afsc ----
                    scps = scpsp.tile([NL, NL * 16], f32, tag="sc")
                    for ch in range(2):
                        for dt in range(NDT):
                            nc.tensor.matmul(
                                scps[:, ch * 512:(ch + 1) * 512],
                                hT[:, dt * NL:(dt + 1) * NL],
                                afsc[:, dt * 1024 + ch * 512: dt * 1024 + (ch + 1) * 512],
                                start=(dt == 0), stop=(dt == NDT - 1),
                            )
                    # diag extract: mask then reduce over n'
                    msk = tp.tile([NL, NL * 16], f32, tag="msk")
                    nc.vector.tensor_mul(msk[:], scps[:], mdiag[:])
                    sc = tp.tile([NL, 16], f32, tag="sc16")
                    nc.vector.tensor_reduce(
                        sc[:],
                        msk[:, :].rearrange("p (np k) -> p k np", k=16),
                        axis=AX.X, op=OP.add,
                    )
                    # exp (softmax numerator), scale folds the 1/sqrt(H)
                    em = tp.tile([NL, 16], f32, tag="em")
                    nc.scalar.activation(em[:], sc[:], AF.Exp, scale=SCALE)

                    # reshape [64,16] -> [128,8] via DRAM roundtrip
                    nc.sync.dma_start(m_dram[:, :], em[:])
                    colv = tp.tile([128, 8], f32, tag="colv")
                    nc.sync.dma_start(
                        colv[:],
                        m_dram[:, :].rearrange("(g s) k -> (s k) g", g=NG),
                    )
                    # per-sample sums over the 16-partition runs + reciprocal
                    smps = smpsp.tile([8, 8], f32, tag="sm")
                    nc.tensor.matmul(smps[:], bdon[:], colv[:], start=True, stop=True)
                    rsg = tp.tile([8, 8], f32, tag="rsg")
                    nc.vector.reciprocal(rsg[:], smps[:])
                    rbps = smpsp.tile([128, 8], f32, tag="sm")
                    nc.tensor.matmul(rbps[:], bdonT[:], rsg[:], start=True, stop=True)
                    rb = tp.tile([128, 8], f32, tag="rb")
                    nc.vector.tensor_copy(rb[:], rbps[:])

                    # BD matrix: normalized softmax in block-diag layout
                    bd = tp.tile([128, NL], bf16, tag="bd")
                    for g in range(NG):
                        nc.vector.tensor_scalar(
                            bd[:, g * 8:(g + 1) * 8],
                            bdm[:, g * 8:(g + 1) * 8],
                            colv[:, g:g + 1],
                            rb[:, g:g + 1],
                            op0=OP.mult, op1=OP.mult,
                        )

                    # ---- attn: attnT[d, n] via block-diag matmuls ----
                    atps = atpsp.tile([128, NDT * NL], f32, tag="at")
                    for dt in range(NDT):
                        for g in range(NG):
                            nc.tensor.matmul(
                                atps[:, dt * NL + g * 8: dt * NL + (g + 1) * 8],
                                afbd[:, (g * NDT + dt) * 128:(g * NDT + dt + 1) * 128],
                                bd[:, g * 8:(g + 1) * 8],
                                start=True, stop=True,
                            )
                    att = tp.tile([128, NDT * NL], bf16, tag="att")
                    nc.vector.tensor_copy(att[:], atps[:])

                    # ---- v halves: h@Wh + attn@Wattn (+xW) ----
                    h_new = hp.tile([NL, H], f32, tag="h")
                    c_new = cp.tile([NL, H], f32, tag="c2")
                    for half in range(2):
                        # xW slice for this half: cols jc*512, jc = half,half+2,..
                        xwt = xwp.tile([NL, 2048], bf16, tag="xwt")
                        nc.gpsimd.dma_start(
                            xwt[:],
                            xw_dram[t * NL:(t + 1) * NL, :]
                            .rearrange("p (q h i) -> p q h i", q=4, h=2)[:, :, half, :],
                        )
                        vps = vpsp.tile([NL, 4 * 512], f32, tag="v")
                        for q in range(4):
                            jc = 2 * q + half
                            for dt in range(NDT):
                                nc.tensor.matmul(
                                    vps[:, q * 512:(q + 1) * 512],
                                    hT[:, dt * NL:(dt + 1) * NL],
                                    whs[:, dt * J + jc * 512: dt * J + (jc + 1) * 512],
                                    start=(dt == 0), stop=False,
                                )
                            for dt in range(NDT):
                                nc.tensor.matmul(
                                    vps[:, q * 512:(q + 1) * 512],
                                    att[:, dt * NL:(dt + 1) * NL],
                                    wats[:, dt * J + jc * 512: dt * J + (jc + 1) * 512],
                                    start=False, stop=(dt == NDT - 1),
                                )
                        # add xW for this half
                        nc.vector.tensor_add(vps[:], vps[:], xwt[:])
                        # gates from psum
                        sg = tp.tile([NL, 1536], f32, tag="sg")
                        nc.scalar.activation(sg[:], vps[:, 0:1536], AF.Sigmoid)
                        gg = tp.tile([NL, 512], f32, tag="gg")
                        nc.scalar.activation(gg[:], vps[:, 1536:2048], AF.Tanh)
                        ig = tp.tile([NL, 512], f32, tag="ig")
                        nc.vector.tensor_mul(ig[:], sg[:, 0:512], gg[:])
                        fc = tp.tile([NL, 512], f32, tag="fc")
                        nc.vector.tensor_mul(
                            fc[:], sg[:, 512:1024],
                            c_cur[:, half * 512:(half + 1) * 512],
                        )
                        nc.vector.tensor_add(
                            c_new[:, half * 512:(half + 1) * 512], ig[:], fc[:]
                        )
                        tch = tp.tile([NL, 512], f32, tag="tch")
                        nc.scalar.activation(
                            tch[:], c_new[:, half * 512:(half + 1) * 512], AF.Tanh
                        )
                        nc.vector.tensor_mul(
                            h_new[:, half * 512:(half + 1) * 512],
                            sg[:, 1024:1536], tch[:],
                        )

                    # write output
                    nc.sync.dma_start(hs_out[:, t, :], h_new[:])

                    # transpose h -> hT for next step
                    if t < T - 1:
                        trps = scpsp.tile([128, NDT * NL], f32, tag="sc")
                        for dt in range(NDT):
                            nc.tensor.transpose(
                                trps[:, dt * NL:(dt + 1) * NL],
                                h_new[:, dt * 128:(dt + 1) * 128],
                                idnt[:],
                            )
                        hT = st.tile([128, NDT * NL], bf16, tag="hTn")
                        nc.vector.tensor_copy(hT[:], trps[:])
                    c_cur = c_new

    _split_waits(nc, mybir)
    nc.finalize()
    return nc


def _split_waits(nc, mybir):
    """Walrus codegen caps sync-wait commands per instruction (1 for DMA
    pseudo-instructions, ~2 for compute). Tile's sem-assignment can emit
    more. Hoist excess waits onto same-engine NoOps inserted just before
    the instruction — sem waits are monotonic so waiting earlier on the
    same engine is always safe, and a NoOp has no side effects.
    """
    nsplit = 0
    for f in nc.m.functions:
        for b in f.blocks:
            il = b.instructions
            out = []
            changed = False
            for inst in il:
                si = getattr(inst, "sync_info", None)
                waits = list(si.on_wait) if si is not None and si.on_wait else []
                limit = 1
                if len(waits) > limit:
                    extra, keep = waits[:-limit], waits[-limit:]
                    for i in range(0, len(extra), 1):
                        out.append(mybir.InstNoOp(
                            name=f"{inst.name}_ws{i}",
                            engine=inst.engine,
                            ins=[], outs=[],
                            sync_info=mybir.SyncInfo(
                                on_wait=extra[i:i + 1], on_update=[]
                            ),
                        ))
                        nsplit += 1
                    inst.sync_info = mybir.SyncInfo(
                        on_wait=keep, on_update=list(si.on_update)
                    )
                    changed = True
                out.append(inst)
            if changed:
                b.instructions = out
    return nsplit


def _prep_inputs(x, A, Wx, Wh, Wattn, b):
    """Host-side layout prep. Returns list of 8 per-core input dicts."""
    x = np.asarray(x, np.float32)
    A = np.asarray(A, np.float32)
    Wx = np.asarray(Wx, np.float32)
    Wh = np.asarray(Wh, np.float32)
    Wattn = np.asarray(Wattn, np.float32)
    b = np.asarray(b, np.float32)

    def dt_split(w):  # [D, J] f32 -> [NDT, 128, J] bf16
        return np.ascontiguousarray(
            w.reshape(NDT, 128, J).astype(bfloat16)
        )

    wx_s = dt_split(Wx)
    wh_s = dt_split(Wh)
    wat_s = dt_split(Wattn)
    bias_s = np.ascontiguousarray(b.reshape(1, J).astype(bfloat16))
    ones_s = np.ones((1, 128), bfloat16)
    mdiag = np.repeat(np.eye(NL, dtype=np.float32), 16, axis=1).astype(bfloat16)
    bdones = np.kron(np.eye(8, dtype=np.float32), np.ones((16, 1), np.float32))
    bdonesT = np.ascontiguousarray(bdones.T)
    bdmask = bdones[:, np.arange(NL) % 8].astype(bfloat16)
    ident = np.eye(NL, dtype=np.float32)

    Af = A.reshape(N, H, 16)
    h0_full = Af.mean(axis=2)  # [N, H] f32

    maps = []
    for c in range(NCORES):
        sl = slice(c * NL, (c + 1) * NL)
        xc = x[sl]                       # [64, 32, 1024]
        Afc = Af[sl]                     # [64, 1024, 16]
        h0 = h0_full[sl]                 # [64, 1024]

        xT = np.ascontiguousarray(
            xc.transpose(2, 1, 0).reshape(NDT, 128, T * NL).astype(bfloat16)
        )
        afsc = np.ascontiguousarray(
            Afc.transpose(1, 0, 2).reshape(NDT, 128, NL * 16).astype(bfloat16)
        )
        # afbd[(g,dt)][16s+k, dd] = Af[8g+s, 128dt+dd, k]
        afbd = np.ascontiguousarray(
            Afc.reshape(NG, 8, NDT, 128, 16)
            .transpose(0, 2, 1, 4, 3)      # [g, dt, s, k, dd]
            .reshape(NG * NDT, 128, 128)
            .astype(bfloat16)
        )
        # hT0[dd, dt*NL+n] = h0[n, 128dt+dd]
        hT0 = np.ascontiguousarray(
            h0.T.reshape(NDT, 128, NL).transpose(1, 0, 2).reshape(128, NDT * NL)
            .astype(bfloat16)
        )
        maps.append({
            "xT": xT, "Wx": wx_s, "Wh": wh_s, "Wat": wat_s,
            "afsc": afsc, "afbd": afbd, "mdiag": mdiag,
            "bdones": bdones, "bdonesT": bdonesT, "bdmask": bdmask,
            "ident": ident, "hT0": hT0,
            "c0": np.ascontiguousarray(h0),
            "bias": bias_s, "onescol": ones_s,
        })
    return maps


def kernel(x, A, Wx, Wh, Wattn, b, trace=False, trace_kwargs=None):
    from concourse import bass_utils

    if "nc" not in _CACHE:
        _CACHE["nc"] = _build()
    nc = _CACHE["nc"]

    in_maps = _prep_inputs(x, A, Wx, Wh, Wattn, b)
    kwargs = {}
    if trace:
        kwargs["trace"] = True
        kwargs["trace_kwargs"] = trace_kwargs or {}
    res = bass_utils.run_bass_kernel_spmd(
        nc, in_maps, core_ids=list(range(NCORES)), **kwargs
    )
    hs = np.concatenate([r["hs"] for r in res.results], axis=0)
    if trace:
        _CACHE["last_results"] = res
    return np.asarray(hs, np.float32)


if __name__ == "__main__":
    rng = np.random.default_rng(0)
    x = rng.standard_normal((N, T, D), dtype=np.float32)
    A = rng.standard_normal((N, H, 4, 4), dtype=np.float32)
    Wx = rng.standard_normal((D, J), dtype=np.float32) / np.sqrt(D)
    Wh = rng.standard_normal((H, J), dtype=np.float32) / np.sqrt(H)
    Wattn = rng.standard_normal((H, J), dtype=np.float32) / np.sqrt(H)
    b = np.zeros((J,), np.float32)
    out = kernel(x=x, A=A, Wx=Wx, Wh=Wh, Wattn=Wattn, b=b)
    print("out", out.shape, out.dtype, float(np.abs(out).mean()))


# revision 35
# speedup vs baseline: 13.8627x; 13.8627x over previous
"""AttentionLSTM Trainium2 kernel.

N=512, T=32, D=1024, H=1024. 8-way data parallel over batch (64 rows/core).

Per-core algorithm:
  Phase A: xW[(t,n), 4H] = x @ Wx + b  (big efficient matmul, staged to DRAM bf16)
  Loop t = 0..31:
    scores  : PE diag-trick  psum[n,(n',k)] = h @ Af_flat, mask+reduce -> [n,16]
    softmax : exp on ACT (scale 1/32), reshape via DRAM roundtrip to col layout,
              sums + reciprocal broadcast via tiny PE matmuls
    attn    : PE block-diagonal matmuls -> attnT [d, n] directly (transposed)
    v       : psum[n, j] accum of h@Wh + attn@Wattn (h/attn stationary), + xW add
    gates   : sigmoid/tanh on ACT from psum, c/h update on DVE
    hT      : PE transpose h -> stationary layout for next step
"""
import sys
import os

sys.path.insert(0, "/opt/trn_rl_repo")

import numpy as np
from ml_dtypes import bfloat16

N, T, D, H = 512, 32, 1024, 1024
NCORES = 8
NL = N // NCORES          # 64 rows per core
J = 4 * H                 # 4096
NDT = D // 128            # 8 contraction tiles
NG = NL // 8              # 8 attention groups of 8 samples
SCALE = 1.0 / (H ** 0.5)  # 1/32

_CACHE = {}


def _build():
    import concourse.bass as bass
    import concourse.mybir as mybir
    from concourse import tile

    f32 = mybir.dt.float32
    bf16 = mybir.dt.bfloat16
    AF = mybir.ActivationFunctionType
    AX = mybir.AxisListType
    OP = mybir.AluOpType

    nc = bass.Bass()

    # ---- external inputs (per-core) ----
    xT_in = nc.dram_tensor("xT", (NDT, 128, T * NL), bf16, kind="ExternalInput")
    wx_in = nc.dram_tensor("Wx", (NDT, 128, J), bf16, kind="ExternalInput")
    wh_in = nc.dram_tensor("Wh", (NDT, 128, J), bf16, kind="ExternalInput")
    wat_in = nc.dram_tensor("Wat", (NDT, 128, J), bf16, kind="ExternalInput")
    afsc_in = nc.dram_tensor("afsc", (NDT, 128, NL * 16), bf16, kind="ExternalInput")
    afbd_in = nc.dram_tensor("afbd", (NG * NDT, 128, 128), bf16, kind="ExternalInput")
    mdiag_in = nc.dram_tensor("mdiag", (NL, NL * 16), bf16, kind="ExternalInput")
    bdones_in = nc.dram_tensor("bdones", (128, 8), f32, kind="ExternalInput")
    bdonesT_in = nc.dram_tensor("bdonesT", (8, 128), f32, kind="ExternalInput")
    bdmask_in = nc.dram_tensor("bdmask", (128, NL), bf16, kind="ExternalInput")
    ident_in = nc.dram_tensor("ident", (NL, NL), f32, kind="ExternalInput")
    hT0_in = nc.dram_tensor("hT0", (128, NDT * NL), bf16, kind="ExternalInput")
    c0_in = nc.dram_tensor("c0", (NL, H), f32, kind="ExternalInput")
    bias_in = nc.dram_tensor("bias", (1, J), bf16, kind="ExternalInput")
    ones_in = nc.dram_tensor("onescol", (1, 128), bf16, kind="ExternalInput")

    hs_out = nc.dram_tensor("hs", (NL, T, H), f32, kind="ExternalOutput")

    with tile.TileContext(nc) as tc:
        with tc.tile_pool(name="dram", bufs=1, space="DRAM") as dpool:
            xw_dram = dpool.tile([T * NL, J], bf16)
            m_dram = dpool.tile([NL, 16], f32)

            # ---------------- Phase A: xW = x @ Wx + b ----------------
            with (
                tc.tile_pool(name="pa", bufs=1) as pa,
                tc.tile_pool(name="paps", bufs=4, space="PSUM") as paps,
                tc.tile_pool(name="pao", bufs=2) as pao,
            ):
                xTs = pa.tile([128, NDT * T * NL], bf16)     # 32KB/part
                wxs = pa.tile([128, NDT * J], bf16)          # 64KB/part
                bbs = pa.tile([1, J], bf16)
                ones = pa.tile([1, 128], bf16)
                for dt in range(NDT):
                    nc.sync.dma_start(xTs[:, dt * 2048:(dt + 1) * 2048], xT_in[dt])
                    nc.sync.dma_start(wxs[:, dt * J:(dt + 1) * J], wx_in[dt])
                nc.sync.dma_start(bbs[:], bias_in[:, :])
                nc.sync.dma_start(ones[:], ones_in[:, :])

                for m in range(T * NL // 128):   # 16 row-tiles of (t,n)
                    ob = pao.tile([128, J], bf16, tag="pao")
                    for jc in range(J // 512):   # 8
                        ps = paps.tile([128, 512], f32, tag="paps")
                        for dt in range(NDT):
                            nc.tensor.matmul(
                                ps[:],
                                xTs[:, dt * 2048 + m * 128: dt * 2048 + (m + 1) * 128],
                                wxs[:, dt * J + jc * 512: dt * J + (jc + 1) * 512],
                                start=(dt == 0), stop=False,
                            )
                        nc.tensor.matmul(
                            ps[:], ones[:], bbs[:, jc * 512:(jc + 1) * 512],
                            start=False, stop=True,
                        )
                        nc.scalar.copy(ob[:, jc * 512:(jc + 1) * 512], ps[:])
                    nc.gpsimd.dma_start(xw_dram[m * 128:(m + 1) * 128, :], ob[:])

            # ---------------- Main recurrent loop ----------------
            with (
                tc.tile_pool(name="wp", bufs=1) as wp,
                tc.tile_pool(name="st", bufs=1) as st,
                tc.tile_pool(name="hp", bufs=1) as hp,
                tc.tile_pool(name="cp", bufs=2) as cp,
                tc.tile_pool(name="tp", bufs=1) as tp,
                tc.tile_pool(name="xwp", bufs=2) as xwp,
                tc.tile_pool(name="vps", bufs=1, space="PSUM") as vpsp,
                tc.tile_pool(name="scps", bufs=1, space="PSUM") as scpsp,
                tc.tile_pool(name="atps", bufs=1, space="PSUM") as atpsp,
                tc.tile_pool(name="smps", bufs=1, space="PSUM") as smpsp,
            ):
                whs = wp.tile([128, NDT * J], bf16)      # 64KB/part
                wats = wp.tile([128, NDT * J], bf16)     # 64KB/part
                afsc = wp.tile([128, NDT * NL * 16], bf16)  # 16KB/part
                afbd = wp.tile([128, NG * NDT * 128], bf16)  # 16KB/part
                mdiag = wp.tile([NL, NL * 16], bf16)
                bdon = wp.tile([128, 8], f32)
                bdonT = wp.tile([8, 128], f32)
                bdm = wp.tile([128, NL], bf16)
                idnt = wp.tile([NL, NL], f32)

                for dt in range(NDT):
                    nc.sync.dma_start(whs[:, dt * J:(dt + 1) * J], wh_in[dt])
                    nc.sync.dma_start(wats[:, dt * J:(dt + 1) * J], wat_in[dt])
                    nc.sync.dma_start(afsc[:, dt * 1024:(dt + 1) * 1024], afsc_in[dt])
                for gd in range(NG * NDT):
                    nc.sync.dma_start(afbd[:, gd * 128:(gd + 1) * 128], afbd_in[gd])
                nc.sync.dma_start(mdiag[:], mdiag_in[:, :])
                nc.sync.dma_start(bdon[:], bdones_in[:, :])
                nc.sync.dma_start(bdonT[:], bdonesT_in[:, :])
                nc.sync.dma_start(bdm[:], bdmask_in[:, :])
                nc.sync.dma_start(idnt[:], ident_in[:, :])

                # initial state
                hT = st.tile([128, NDT * NL], bf16, tag="hT0")
                nc.sync.dma_start(hT[:], hT0_in[:, :])
                c_cur = cp.tile([NL, H], f32, tag="c2")
                nc.sync.dma_start(c_cur[:], c0_in[:, :])

                for t in range(T):
                    # ---- scores: psum[n, (n',k)] += hT.T @ afsc ----
                    scps = scpsp.tile([NL, NL * 16], f32, tag="sc")
                    for ch in range(2):
                        for dt in range(NDT):
                            nc.tensor.matmul(
                                scps[:, ch * 512:(ch + 1) * 512],
                                hT[:, dt * NL:(dt + 1) * NL],
                                afsc[:, dt * 1024 + ch * 512: dt * 1024 + (ch + 1) * 512],
                                start=(dt == 0), stop=(dt == NDT - 1),
                            )
                    # diag extract: mask then reduce over n'
                    msk = tp.tile([NL, NL * 16], f32, tag="msk")
                    nc.vector.tensor_mul(msk[:], scps[:], mdiag[:])
                    sc = tp.tile([NL, 16], f32, tag="sc16")
                    nc.vector.tensor_reduce(
                        sc[:],
                        msk[:, :].rearrange("p (np k) -> p k np", k=16),
                        axis=AX.X, op=OP.add,
                    )
                    # exp (softmax numerator), scale folds the 1/sqrt(H)
                    em = tp.tile([NL, 16], f32, tag="em")
                    nc.scalar.activation(em[:], sc[:], AF.Exp, scale=SCALE)

                    # reshape [64,16] -> [128,8] via DRAM roundtrip
                    nc.sync.dma_start(m_dram[:, :], em[:])
                    colv = tp.tile([128, 8], f32, tag="colv")
                    nc.sync.dma_start(
                        colv[:],
                        m_dram[:, :].rearrange("(g s) k -> (s k) g", g=NG),
                    )

                    # PE gap fillers while the softmax chain runs
                    # per-sample sums over the 16-partition runs + reciprocal
                    smps = smpsp.tile([8, 8], f32, tag="sm")
                    nc.tensor.matmul(smps[:], bdon[:], colv[:], start=True, stop=True)
                    rsg = tp.tile([8, 8], f32, tag="rsg")
                    nc.vector.reciprocal(rsg[:], smps[:])
                    rbps = smpsp.tile([128, 8], f32, tag="sm")
                    nc.tensor.matmul(rbps[:], bdonT[:], rsg[:], start=True, stop=True)
                    rb = tp.tile([128, 8], f32, tag="rb")
                    nc.vector.tensor_copy(rb[:], rbps[:])

                    # BD matrix: normalized softmax in block-diag layout
                    bd = tp.tile([128, NL], bf16, tag="bd")
                    for g in range(NG):
                        nc.vector.tensor_scalar(
                            bd[:, g * 8:(g + 1) * 8],
                            bdm[:, g * 8:(g + 1) * 8],
                            colv[:, g:g + 1],
                            rb[:, g:g + 1],
                            op0=OP.mult, op1=OP.mult,
                        )

                    # ---- attn: attnT[d, n] via block-diag matmuls ----
                    atps = atpsp.tile([128, NDT * NL], f32, tag="at")
                    for dt in range(NDT):
                        for g in range(NG):
                            nc.tensor.matmul(
                                atps[:, dt * NL + g * 8: dt * NL + (g + 1) * 8],
                                afbd[:, (g * NDT + dt) * 128:(g * NDT + dt + 1) * 128],
                                bd[:, g * 8:(g + 1) * 8],
                                start=True, stop=True,
                            )
                    att = tp.tile([128, NDT * NL], bf16, tag="att")
                    nc.vector.tensor_copy(att[:], atps[:])

                    # ---- v halves: h@Wh + attn@Wattn (+xW) ----
                    h_new = hp.tile([NL, H], f32, tag="h")
                    c_new = cp.tile([NL, H], f32, tag="c2")
                    for half in range(2):
                        # xW slice for this half: cols jc*512, jc = half,half+2,..
                        xwt = xwp.tile([NL, 2048], bf16, tag="xwt")
                        nc.gpsimd.dma_start(
                            xwt[:],
                            xw_dram[t * NL:(t + 1) * NL, :]
                            .rearrange("p (q h i) -> p q h i", q=4, h=2)[:, :, half, :],
                        )
                        vps = vpsp.tile([NL, 4 * 512], f32, tag="v")
                        for q in range(4):
                            jc = 2 * q + half
                            for dt in range(NDT):
                                nc.tensor.matmul(
                                    vps[:, q * 512:(q + 1) * 512],
                                    hT[:, dt * NL:(dt + 1) * NL],
                                    whs[:, dt * J + jc * 512: dt * J + (jc + 1) * 512],
                                    start=(dt == 0), stop=False,
                                )
                            for dt in range(NDT):
                                nc.tensor.matmul(
                                    vps[:, q * 512:(q + 1) * 512],
                                    att[:, dt * NL:(dt + 1) * NL],
                                    wats[:, dt * J + jc * 512: dt * J + (jc + 1) * 512],
                                    start=False, stop=(dt == NDT - 1),
                                )
                        # add xW for this half
                        nc.vector.tensor_add(vps[:], vps[:], xwt[:])
                        # gates from psum
                        sg = tp.tile([NL, 1536], f32, tag="sg")
                        nc.scalar.activation(sg[:], vps[:, 0:1536], AF.Sigmoid)
                        gg = tp.tile([NL, 512], f32, tag="gg")
                        nc.scalar.activation(gg[:], vps[:, 1536:2048], AF.Tanh)
                        ig = tp.tile([NL, 512], f32, tag="ig")
                        nc.vector.tensor_mul(ig[:], sg[:, 0:512], gg[:])
                        fc = tp.tile([NL, 512], f32, tag="fc")
                        nc.vector.tensor_mul(
                            fc[:], sg[:, 512:1024],
                            c_cur[:, half * 512:(half + 1) * 512],
                        )
                        nc.vector.tensor_add(
                            c_new[:, half * 512:(half + 1) * 512], ig[:], fc[:]
                        )
                        tch = tp.tile([NL, 512], f32, tag="tch")
                        nc.scalar.activation(
                            tch[:], c_new[:, half * 512:(half + 1) * 512], AF.Tanh
                        )
                        nc.vector.tensor_mul(
                            h_new[:, half * 512:(half + 1) * 512],
                            sg[:, 1024:1536], tch[:],
                        )

                    # write output
                    nc.sync.dma_start(hs_out[:, t, :], h_new[:])

                    # transpose h -> hT for next step
                    if t < T - 1:
                        trps = scpsp.tile([128, NDT * NL], f32, tag="sc")
                        for dt in range(NDT):
                            nc.tensor.transpose(
                                trps[:, dt * NL:(dt + 1) * NL],
                                h_new[:, dt * 128:(dt + 1) * 128],
                                idnt[:],
                            )
                        hT = st.tile([128, NDT * NL], bf16, tag="hTn")
                        nc.vector.tensor_copy(hT[:], trps[:])
                    c_cur = c_new

    _split_waits(nc, mybir)
    nc.finalize()
    return nc


def _split_waits(nc, mybir):
    """Walrus codegen caps sync-wait commands per instruction (1 for DMA
    pseudo-instructions, ~2 for compute). Tile's sem-assignment can emit
    more. Hoist excess waits onto same-engine NoOps inserted just before
    the instruction — sem waits are monotonic so waiting earlier on the
    same engine is always safe, and a NoOp has no side effects.
    """
    nsplit = 0
    for f in nc.m.functions:
        for b in f.blocks:
            il = b.instructions
            out = []
            changed = False
            for inst in il:
                si = getattr(inst, "sync_info", None)
                waits = list(si.on_wait) if si is not None and si.on_wait else []
                limit = 1
                if len(waits) > limit:
                    extra, keep = waits[:-limit], waits[-limit:]
                    for i in range(0, len(extra), 1):
                        out.append(mybir.InstNoOp(
                            name=f"{inst.name}_ws{i}",
                            engine=inst.engine,
                            ins=[], outs=[],
                            sync_info=mybir.SyncInfo(
                                on_wait=extra[i:i + 1], on_update=[]
                            ),
                        ))
                        nsplit += 1
                    inst.sync_info = mybir.SyncInfo(
                        on_wait=keep, on_update=list(si.on_update)
                    )
                    changed = True
                out.append(inst)
            if changed:
                b.instructions = out
    return nsplit


def _prep_inputs(x, A, Wx, Wh, Wattn, b):
    """Host-side layout prep. Returns list of 8 per-core input dicts."""
    x = np.asarray(x, np.float32)
    A = np.asarray(A, np.float32)
    Wx = np.asarray(Wx, np.float32)
    Wh = np.asarray(Wh, np.float32)
    Wattn = np.asarray(Wattn, np.float32)
    b = np.asarray(b, np.float32)

    def dt_split(w):  # [D, J] f32 -> [NDT, 128, J] bf16
        return np.ascontiguousarray(
            w.reshape(NDT, 128, J).astype(bfloat16)
        )

    wx_s = dt_split(Wx)
    wh_s = dt_split(Wh)
    wat_s = dt_split(Wattn)
    bias_s = np.ascontiguousarray(b.reshape(1, J).astype(bfloat16))
    ones_s = np.ones((1, 128), bfloat16)
    mdiag = np.repeat(np.eye(NL, dtype=np.float32), 16, axis=1).astype(bfloat16)
    bdones = np.kron(np.eye(8, dtype=np.float32), np.ones((16, 1), np.float32))
    bdonesT = np.ascontiguousarray(bdones.T)
    bdmask = bdones[:, np.arange(NL) % 8].astype(bfloat16)
    ident = np.eye(NL, dtype=np.float32)

    Af = A.reshape(N, H, 16)
    h0_full = Af.mean(axis=2)  # [N, H] f32

    maps = []
    for c in range(NCORES):
        sl = slice(c * NL, (c + 1) * NL)
        xc = x[sl]                       # [64, 32, 1024]
        Afc = Af[sl]                     # [64, 1024, 16]
        h0 = h0_full[sl]                 # [64, 1024]

        xT = np.ascontiguousarray(
            xc.transpose(2, 1, 0).reshape(NDT, 128, T * NL).astype(bfloat16)
        )
        afsc = np.ascontiguousarray(
            Afc.transpose(1, 0, 2).reshape(NDT, 128, NL * 16).astype(bfloat16)
        )
        # afbd[(g,dt)][16s+k, dd] = Af[8g+s, 128dt+dd, k]
        afbd = np.ascontiguousarray(
            Afc.reshape(NG, 8, NDT, 128, 16)
            .transpose(0, 2, 1, 4, 3)      # [g, dt, s, k, dd]
            .reshape(NG * NDT, 128, 128)
            .astype(bfloat16)
        )
        # hT0[dd, dt*NL+n] = h0[n, 128dt+dd]
        hT0 = np.ascontiguousarray(
            h0.T.reshape(NDT, 128, NL).transpose(1, 0, 2).reshape(128, NDT * NL)
            .astype(bfloat16)
        )
        maps.append({
            "xT": xT, "Wx": wx_s, "Wh": wh_s, "Wat": wat_s,
            "afsc": afsc, "afbd": afbd, "mdiag": mdiag,
            "bdones": bdones, "bdonesT": bdonesT, "bdmask": bdmask,
            "ident": ident, "hT0": hT0,
            "c0": np.ascontiguousarray(h0),
            "bias": bias_s, "onescol": ones_s,
        })
    return maps


def kernel(x, A, Wx, Wh, Wattn, b, trace=False, trace_kwargs=None):
    from concourse import bass_utils

    if "nc" not in _CACHE:
        _CACHE["nc"] = _build()
    nc = _CACHE["nc"]

    in_maps = _prep_inputs(x, A, Wx, Wh, Wattn, b)
    kwargs = {}
    if trace:
        kwargs["trace"] = True
        kwargs["trace_kwargs"] = trace_kwargs or {}
    res = bass_utils.run_bass_kernel_spmd(
        nc, in_maps, core_ids=list(range(NCORES)), **kwargs
    )
    hs = np.concatenate([r["hs"] for r in res.results], axis=0)
    if trace:
        _CACHE["last_results"] = res
    return np.asarray(hs, np.float32)


if __name__ == "__main__":
    rng = np.random.default_rng(0)
    x = rng.standard_normal((N, T, D), dtype=np.float32)
    A = rng.standard_normal((N, H, 4, 4), dtype=np.float32)
    Wx = rng.standard_normal((D, J), dtype=np.float32) / np.sqrt(D)
    Wh = rng.standard_normal((H, J), dtype=np.float32) / np.sqrt(H)
    Wattn = rng.standard_normal((H, J), dtype=np.float32) / np.sqrt(H)
    b = np.zeros((J,), np.float32)
    out = kernel(x=x, A=A, Wx=Wx, Wh=Wh, Wattn=Wattn, b=b)
    print("out", out.shape, out.dtype, float(np.abs(out).mean()))


# revision 40
# speedup vs baseline: 19.6162x; 1.4150x over previous
"""AttentionLSTM Trainium2 kernel.

N=512, T=32, D=1024, H=1024. 8-way data parallel over batch (64 rows/core).

Per-core algorithm:
  Phase A: xW[(t,n), 4H] = x @ Wx + b  (big efficient matmul, staged to DRAM bf16)
  Loop t = 0..31:
    scores  : PE diag-trick  psum[n,(n',k)] = h @ Af_flat, mask+reduce -> [n,16]
    softmax : exp on ACT (scale 1/32), reshape via DRAM roundtrip to col layout,
              sums + reciprocal broadcast via tiny PE matmuls
    attn    : PE block-diagonal matmuls -> attnT [d, n] directly (transposed)
    v       : psum[n, j] accum of h@Wh + attn@Wattn (h/attn stationary), + xW add
    gates   : sigmoid/tanh on ACT from psum, c/h update on DVE
    hT      : PE transpose h -> stationary layout for next step
"""
import sys
import os

sys.path.insert(0, "/opt/trn_rl_repo")

import numpy as np
from ml_dtypes import bfloat16

N, T, D, H = 512, 32, 1024, 1024
NCORES = 8
NL = N // NCORES          # 64 rows per core
J = 4 * H                 # 4096
NDT = D // 128            # 8 contraction tiles
NG = NL // 8              # 8 attention groups of 8 samples
SCALE = 1.0 / (H ** 0.5)  # 1/32

_CACHE = {}


def _build():
    import concourse.bass as bass
    import concourse.mybir as mybir
    from concourse import tile

    f32 = mybir.dt.float32
    bf16 = mybir.dt.bfloat16
    AF = mybir.ActivationFunctionType
    AX = mybir.AxisListType
    OP = mybir.AluOpType

    nc = bass.Bass()

    # ---- external inputs (per-core) ----
    xT_in = nc.dram_tensor("xT", (NDT, 128, T * NL), bf16, kind="ExternalInput")
    wx_in = nc.dram_tensor("Wx", (NDT, 128, J), bf16, kind="ExternalInput")
    wh_in = nc.dram_tensor("Wh", (NDT, 128, J), bf16, kind="ExternalInput")
    wat_in = nc.dram_tensor("Wat", (NDT, 128, J), bf16, kind="ExternalInput")
    afsc_in = nc.dram_tensor("afsc", (NDT, 128, NL * 16), bf16, kind="ExternalInput")
    afbd_in = nc.dram_tensor("afbd", (NG * NDT, 128, 128), bf16, kind="ExternalInput")
    mdiag_in = nc.dram_tensor("mdiag", (NL, NL * 16), bf16, kind="ExternalInput")
    bdones_in = nc.dram_tensor("bdones", (128, 8), f32, kind="ExternalInput")
    bdonesT_in = nc.dram_tensor("bdonesT", (8, 128), f32, kind="ExternalInput")
    bdmask_in = nc.dram_tensor("bdmask", (128, NL), bf16, kind="ExternalInput")
    ident_in = nc.dram_tensor("ident", (NL, NL), f32, kind="ExternalInput")
    hT0_in = nc.dram_tensor("hT0", (128, NDT * NL), bf16, kind="ExternalInput")
    c0_in = nc.dram_tensor("c0", (NL, H), f32, kind="ExternalInput")
    bias_in = nc.dram_tensor("bias", (1, J), bf16, kind="ExternalInput")
    ones_in = nc.dram_tensor("onescol", (1, 128), bf16, kind="ExternalInput")

    hs_out = nc.dram_tensor("hs", (NL, T, H), f32, kind="ExternalOutput")

    with tile.TileContext(nc) as tc:
        with tc.tile_pool(name="dram", bufs=1, space="DRAM") as dpool:
            xw_dram = dpool.tile([T * NL, J], bf16)
            m_dram = dpool.tile([NL, 16], f32)

            # ---------------- Phase A: xW = x @ Wx + b ----------------
            with (
                tc.tile_pool(name="pa", bufs=1) as pa,
                tc.tile_pool(name="paps", bufs=4, space="PSUM") as paps,
                tc.tile_pool(name="pao", bufs=2) as pao,
            ):
                xTs = pa.tile([128, NDT * T * NL], bf16)     # 32KB/part
                wxs = pa.tile([128, NDT * J], bf16)          # 64KB/part
                bbs = pa.tile([1, J], bf16)
                ones = pa.tile([1, 128], bf16)
                for dt in range(NDT):
                    nc.sync.dma_start(xTs[:, dt * 2048:(dt + 1) * 2048], xT_in[dt])
                    nc.sync.dma_start(wxs[:, dt * J:(dt + 1) * J], wx_in[dt])
                nc.sync.dma_start(bbs[:], bias_in[:, :])
                nc.sync.dma_start(ones[:], ones_in[:, :])

                for m in range(T * NL // 128):   # 16 row-tiles of (t,n)
                    ob = pao.tile([128, J], bf16, tag="pao")
                    for jc in range(J // 512):   # 8
                        ps = paps.tile([128, 512], f32, tag="paps")
                        for dt in range(NDT):
                            nc.tensor.matmul(
                                ps[:],
                                xTs[:, dt * 2048 + m * 128: dt * 2048 + (m + 1) * 128],
                                wxs[:, dt * J + jc * 512: dt * J + (jc + 1) * 512],
                                start=(dt == 0), stop=False,
                            )
                        nc.tensor.matmul(
                            ps[:], ones[:], bbs[:, jc * 512:(jc + 1) * 512],
                            start=False, stop=True,
                        )
                        nc.scalar.copy(ob[:, jc * 512:(jc + 1) * 512], ps[:])
                    nc.gpsimd.dma_start(xw_dram[m * 128:(m + 1) * 128, :], ob[:])

            # ---------------- Main recurrent loop ----------------
            with (
                tc.tile_pool(name="wp", bufs=1) as wp,
                tc.tile_pool(name="st", bufs=1) as st,
                tc.tile_pool(name="hp", bufs=1) as hp,
                tc.tile_pool(name="cp", bufs=2) as cp,
                tc.tile_pool(name="tp", bufs=1) as tp,
                tc.tile_pool(name="xwp", bufs=2) as xwp,
                tc.tile_pool(name="vq", bufs=3, space="PSUM") as vqp,
                tc.tile_pool(name="atps", bufs=1, space="PSUM") as atpsp,
                tc.tile_pool(name="smps", bufs=1, space="PSUM") as smpsp,
            ):
                whs = wp.tile([128, NDT * J], bf16)      # 64KB/part
                wats = wp.tile([128, NDT * J], bf16)     # 64KB/part
                afsc = wp.tile([128, NDT * NL * 16], bf16)  # 16KB/part
                afbd = wp.tile([128, NG * NDT * 128], bf16)  # 16KB/part
                mdiag = wp.tile([NL, NL * 16], bf16)
                bdon = wp.tile([128, 8], f32)
                bdonT = wp.tile([8, 128], f32)
                bdm = wp.tile([128, NL], bf16)
                idnt = wp.tile([NL, NL], f32)

                for dt in range(NDT):
                    nc.sync.dma_start(whs[:, dt * J:(dt + 1) * J], wh_in[dt])
                    nc.sync.dma_start(wats[:, dt * J:(dt + 1) * J], wat_in[dt])
                    nc.sync.dma_start(afsc[:, dt * 1024:(dt + 1) * 1024], afsc_in[dt])
                for gd in range(NG * NDT):
                    nc.sync.dma_start(afbd[:, gd * 128:(gd + 1) * 128], afbd_in[gd])
                nc.sync.dma_start(mdiag[:], mdiag_in[:, :])
                nc.sync.dma_start(bdon[:], bdones_in[:, :])
                nc.sync.dma_start(bdonT[:], bdonesT_in[:, :])
                nc.sync.dma_start(bdm[:], bdmask_in[:, :])
                nc.sync.dma_start(idnt[:], ident_in[:, :])

                # initial state
                hT = st.tile([128, NDT * NL], bf16, tag="hT0")
                nc.sync.dma_start(hT[:], hT0_in[:, :])
                c_cur = cp.tile([NL, H], f32, tag="c2")
                nc.sync.dma_start(c_cur[:], c0_in[:, :])

                for t in range(T):
                    # ---- scores: psum[n, (n',k)] += hT.T @ afsc ----
                    scps = vqp.tile([NL, NL * 16], f32, tag="vq")
                    for ch in range(2):
                        for dt in range(NDT):
                            nc.tensor.matmul(
                                scps[:, ch * 512:(ch + 1) * 512],
                                hT[:, dt * NL:(dt + 1) * NL],
                                afsc[:, dt * 1024 + ch * 512: dt * 1024 + (ch + 1) * 512],
                                start=(dt == 0), stop=(dt == NDT - 1),
                            )
                    # v psum quarters; [64, 1024] = 2 banks each.
                    # NOTE start=True clears has_written at BANK granularity,
                    # so only the first 256-wide group per bank gets start=True
                    vps_q = [vqp.tile([NL, 1024], f32, tag="vq",
                                      name=f"vqt{t}_{q}")
                             for q in range(4)]
                    xwt = xwp.tile([NL, J], bf16, tag="xwt",
                                   name=f"xwt{t}")
                    nc.gpsimd.dma_start(
                        xwt[:], xw_dram[t * NL:(t + 1) * NL, :])

                    def wh_mms(q):
                        for g4 in range(4):
                            jo = g4 * 1024 + q * 256
                            cs = slice(g4 * 256, (g4 + 1) * 256)
                            for dt in range(NDT):
                                nc.tensor.matmul(
                                    vps_q[q][:, cs],
                                    hT[:, dt * NL:(dt + 1) * NL],
                                    whs[:, dt * J + jo: dt * J + jo + 256],
                                    start=(dt == 0 and g4 % 2 == 0),
                                    stop=False,
                                    skip_group_check=True,
                                )

                    def wat_mms(q):
                        for g4 in range(4):
                            jo = g4 * 1024 + q * 256
                            cs = slice(g4 * 256, (g4 + 1) * 256)
                            for dt in range(NDT):
                                nc.tensor.matmul(
                                    vps_q[q][:, cs],
                                    att[:, dt * NL:(dt + 1) * NL],
                                    wats[:, dt * J + jo: dt * J + jo + 256],
                                    start=False,
                                    stop=(dt == NDT - 1 and g4 % 2 == 1),
                                    skip_group_check=True,
                                )

                    # diag extract: mask then reduce over n'
                    msk = tp.tile([NL, NL * 16], f32, tag="msk")
                    nc.vector.tensor_mul(msk[:], scps[:], mdiag[:])
                    sc = tp.tile([NL, 16], f32, tag="sc16")
                    nc.vector.tensor_reduce(
                        sc[:],
                        msk[:, :].rearrange("p (np k) -> p k np", k=16),
                        axis=AX.X, op=OP.add,
                    )
                    # exp (softmax numerator), scale folds the 1/sqrt(H)
                    em = tp.tile([NL, 16], f32, tag="em")
                    nc.scalar.activation(em[:], sc[:], AF.Exp, scale=SCALE)

                    # reshape [64,16] -> [128,8] via DRAM roundtrip
                    nc.sync.dma_start(m_dram[:, :], em[:])
                    colv = tp.tile([128, 8], f32, tag="colv")
                    nc.sync.dma_start(
                        colv[:],
                        m_dram[:, :].rearrange("(g s) k -> (s k) g", g=NG),
                    )

                    # PE gap fillers while the softmax chain runs
                    wh_mms(0)
                    wh_mms(1)

                    # per-sample sums over the 16-partition runs + reciprocal
                    smps = smpsp.tile([8, 8], f32, tag="sm")
                    nc.tensor.matmul(smps[:], bdon[:], colv[:], start=True, stop=True)
                    rsg = tp.tile([8, 8], f32, tag="rsg")
                    nc.vector.reciprocal(rsg[:], smps[:])
                    rbps = smpsp.tile([128, 8], f32, tag="sm")
                    nc.tensor.matmul(rbps[:], bdonT[:], rsg[:], start=True, stop=True)
                    rb = tp.tile([128, 8], f32, tag="rb")
                    nc.vector.tensor_copy(rb[:], rbps[:])

                    # BD matrix: normalized softmax in block-diag layout
                    bd = tp.tile([128, NL], bf16, tag="bd")
                    for g in range(NG):
                        nc.vector.tensor_scalar(
                            bd[:, g * 8:(g + 1) * 8],
                            bdm[:, g * 8:(g + 1) * 8],
                            colv[:, g:g + 1],
                            rb[:, g:g + 1],
                            op0=OP.mult, op1=OP.mult,
                        )

                    # ---- attn: attnT[d, n] via block-diag matmuls ----
                    atps = atpsp.tile([128, NDT * NL], f32, tag="at")
                    for dt in range(NDT):
                        for g in range(NG):
                            nc.tensor.matmul(
                                atps[:, dt * NL + g * 8: dt * NL + (g + 1) * 8],
                                afbd[:, (g * NDT + dt) * 128:(g * NDT + dt + 1) * 128],
                                bd[:, g * 8:(g + 1) * 8],
                                start=True, stop=True,
                            )
                    att = tp.tile([128, NDT * NL], bf16, tag="att")
                    nc.vector.tensor_copy(att[:], atps[:])

                    # remaining Wh quarters
                    wh_mms(2)
                    wh_mms(3)

                    # ---- v quarters: finish with Wattn, add xW, gates ----
                    h_new = hp.tile([NL, H], f32, tag="h")
                    c_new = cp.tile([NL, H], f32, tag="c2")
                    for q in range(4):
                        wat_mms(q)
                        vps = vps_q[q]
                        # add xW for this quarter (strided cols g*1024+q*256)
                        xw_view = xwt[:, :].rearrange(
                            "p (g q i) -> p g q i", g=4, q=4)[:, :, q, :]
                        vv = vps[:, :].rearrange("p (g i) -> p g i", g=4)
                        nc.vector.tensor_add(vv, vv, xw_view)
                        # gates from psum; quarter covers h-dims [256q, 256q+256)
                        hsl = slice(q * 256, (q + 1) * 256)
                        sg = tp.tile([NL, 768], f32, tag="sg")
                        nc.scalar.activation(sg[:], vps[:, 0:768], AF.Sigmoid)
                        gg = tp.tile([NL, 256], f32, tag="gg")
                        nc.scalar.activation(gg[:], vps[:, 768:1024], AF.Tanh)
                        ig = tp.tile([NL, 256], f32, tag="ig")
                        nc.vector.tensor_mul(ig[:], sg[:, 0:256], gg[:])
                        fc = tp.tile([NL, 256], f32, tag="fc")
                        nc.vector.tensor_mul(fc[:], sg[:, 256:512], c_cur[:, hsl])
                        nc.vector.tensor_add(c_new[:, hsl], ig[:], fc[:])
                        tch = tp.tile([NL, 256], f32, tag="tch")
                        nc.scalar.activation(tch[:], c_new[:, hsl], AF.Tanh)
                        nc.vector.tensor_mul(
                            h_new[:, hsl], sg[:, 512:768], tch[:]
                        )

                    # write output
                    nc.sync.dma_start(hs_out[:, t, :], h_new[:])

                    # transpose h -> hT for next step
                    if t < T - 1:
                        trps = vqp.tile([128, NDT * NL], f32, tag="vq")
                        for dt in range(NDT):
                            nc.tensor.transpose(
                                trps[:, dt * NL:(dt + 1) * NL],
                                h_new[:, dt * 128:(dt + 1) * 128],
                                idnt[:],
                            )
                        hT = st.tile([128, NDT * NL], bf16, tag="hTn")
                        nc.vector.tensor_copy(hT[:], trps[:])
                    c_cur = c_new

    _split_waits(nc, mybir)
    nc.finalize()
    return nc


def _split_waits(nc, mybir):
    """Walrus codegen caps sync-wait commands per instruction (1 for DMA
    pseudo-instructions, ~2 for compute). Tile's sem-assignment can emit
    more. Hoist excess waits onto same-engine NoOps inserted just before
    the instruction — sem waits are monotonic so waiting earlier on the
    same engine is always safe, and a NoOp has no side effects.
    """
    nsplit = 0
    for f in nc.m.functions:
        for b in f.blocks:
            il = b.instructions
            out = []
            changed = False
            for inst in il:
                si = getattr(inst, "sync_info", None)
                waits = list(si.on_wait) if si is not None and si.on_wait else []
                limit = 1
                if len(waits) > limit:
                    extra, keep = waits[:-limit], waits[-limit:]
                    for i in range(0, len(extra), 1):
                        out.append(mybir.InstNoOp(
                            name=f"{inst.name}_ws{i}",
                            engine=inst.engine,
                            ins=[], outs=[],
                            sync_info=mybir.SyncInfo(
                                on_wait=extra[i:i + 1], on_update=[]
                            ),
                        ))
                        nsplit += 1
                    inst.sync_info = mybir.SyncInfo(
                        on_wait=keep, on_update=list(si.on_update)
                    )
                    changed = True
                out.append(inst)
            if changed:
                b.instructions = out
    return nsplit


def _prep_inputs(x, A, Wx, Wh, Wattn, b):
    """Host-side layout prep. Returns list of 8 per-core input dicts."""
    x = np.asarray(x, np.float32)
    A = np.asarray(A, np.float32)
    Wx = np.asarray(Wx, np.float32)
    Wh = np.asarray(Wh, np.float32)
    Wattn = np.asarray(Wattn, np.float32)
    b = np.asarray(b, np.float32)

    def dt_split(w):  # [D, J] f32 -> [NDT, 128, J] bf16
        return np.ascontiguousarray(
            w.reshape(NDT, 128, J).astype(bfloat16)
        )

    wx_s = dt_split(Wx)
    wh_s = dt_split(Wh)
    wat_s = dt_split(Wattn)
    bias_s = np.ascontiguousarray(b.reshape(1, J).astype(bfloat16))
    ones_s = np.ones((1, 128), bfloat16)
    mdiag = np.repeat(np.eye(NL, dtype=np.float32), 16, axis=1).astype(bfloat16)
    bdones = np.kron(np.eye(8, dtype=np.float32), np.ones((16, 1), np.float32))
    bdonesT = np.ascontiguousarray(bdones.T)
    bdmask = bdones[:, np.arange(NL) % 8].astype(bfloat16)
    ident = np.eye(NL, dtype=np.float32)

    Af = A.reshape(N, H, 16)
    h0_full = Af.mean(axis=2)  # [N, H] f32

    maps = []
    for c in range(NCORES):
        sl = slice(c * NL, (c + 1) * NL)
        xc = x[sl]                       # [64, 32, 1024]
        Afc = Af[sl]                     # [64, 1024, 16]
        h0 = h0_full[sl]                 # [64, 1024]

        xT = np.ascontiguousarray(
            xc.transpose(2, 1, 0).reshape(NDT, 128, T * NL).astype(bfloat16)
        )
        afsc = np.ascontiguousarray(
            Afc.transpose(1, 0, 2).reshape(NDT, 128, NL * 16).astype(bfloat16)
        )
        # afbd[(g,dt)][16s+k, dd] = Af[8g+s, 128dt+dd, k]
        afbd = np.ascontiguousarray(
            Afc.reshape(NG, 8, NDT, 128, 16)
            .transpose(0, 2, 1, 4, 3)      # [g, dt, s, k, dd]
            .reshape(NG * NDT, 128, 128)
            .astype(bfloat16)
        )
        # hT0[dd, dt*NL+n] = h0[n, 128dt+dd]
        hT0 = np.ascontiguousarray(
            h0.T.reshape(NDT, 128, NL).transpose(1, 0, 2).reshape(128, NDT * NL)
            .astype(bfloat16)
        )
        maps.append({
            "xT": xT, "Wx": wx_s, "Wh": wh_s, "Wat": wat_s,
            "afsc": afsc, "afbd": afbd, "mdiag": mdiag,
            "bdones": bdones, "bdonesT": bdonesT, "bdmask": bdmask,
            "ident": ident, "hT0": hT0,
            "c0": np.ascontiguousarray(h0),
            "bias": bias_s, "onescol": ones_s,
        })
    return maps


def kernel(x, A, Wx, Wh, Wattn, b, trace=False, trace_kwargs=None):
    from concourse import bass_utils

    if "nc" not in _CACHE:
        _CACHE["nc"] = _build()
    nc = _CACHE["nc"]

    in_maps = _prep_inputs(x, A, Wx, Wh, Wattn, b)
    kwargs = {}
    if trace:
        kwargs["trace"] = True
        kwargs["trace_kwargs"] = trace_kwargs or {}
    res = bass_utils.run_bass_kernel_spmd(
        nc, in_maps, core_ids=list(range(NCORES)), **kwargs
    )
    hs = np.concatenate([r["hs"] for r in res.results], axis=0)
    if trace:
        _CACHE["last_results"] = res
    return np.asarray(hs, np.float32)


if __name__ == "__main__":
    rng = np.random.default_rng(0)
    x = rng.standard_normal((N, T, D), dtype=np.float32)
    A = rng.standard_normal((N, H, 4, 4), dtype=np.float32)
    Wx = rng.standard_normal((D, J), dtype=np.float32) / np.sqrt(D)
    Wh = rng.standard_normal((H, J), dtype=np.float32) / np.sqrt(H)
    Wattn = rng.standard_normal((H, J), dtype=np.float32) / np.sqrt(H)
    b = np.zeros((J,), np.float32)
    out = kernel(x=x, A=A, Wx=Wx, Wh=Wh, Wattn=Wattn, b=b)
    print("out", out.shape, out.dtype, float(np.abs(out).mean()))
